# revision 17
# baseline (speedup 1.0000x reference)
"""Trainium2 Bass kernel for nn_Network24 (QuasiPoly 2->2 layer + Network4Infra head).

Math per row (powers are 1.0 in this problem's inputs):
    h0 = sigmoid(w00*x0 + w01*x1 + b0)
    h1 = sigmoid(w10*x0 + w11*x1 + b1)
    out = sigmoid(v),  v = q*(h0 + c0)*(h1 + c1) + cfin,  q = p1*p2

Design (v3):
  * Final sigmoid: over the full input hypercube x in [0,1)^2, v lies in a
    narrow interval ([0.265, 0.496] for the staged weights) where sigmoid is
    nearly affine.  Replace it with a minimax linear fit alpha*v + beta
    (max err ~1.5e-4), folding alpha*q into the device constants and the
    remaining offset D into the host-side f32 dequant of the bf16 outputs.
  * Linear layer on the TensorEngine: batch rows are split into 64 chunks
    per core; an SBUF tile [128, F] holds x0-chunks on partitions 0-63 and
    x1-chunks on partitions 64-127.  One resident block-diagonal [128,128]
    bf16 weight matrix maps this to PSUM z-tiles with z0 on partitions 0-63
    and z1 on partitions 64-127.  This removes all FMA work from DVE
    (scalar_tensor_tensor has no DVE fast modes - measured 1x).
  * Both sigmoids in ONE ACT pass per group: sigma(z + bias) with a
    per-partition bias vector (b0 on parts 0-63, b1 on parts 64-127),
    reading 4 PSUM banks per instruction, writing bf16 to SBUF.
  * Tail on DVE in fast modes, pairing two groups into full-width tiles
    using partition-rebasing tensor_scalar ops (verified legal for
    single-tensor-input ops):
        t0 = h0 + c0            (tensor_scalar_add, bf16 4x)
        g1 = Aq*h1 + Bq         (tensor_scalar affine, bf16 4x)
        yo = t0 * g1            (tensor_tensor, bf16 2x, 128 partitions)
  * I/O: input columns quantized to fp8-e4m3 on host (verified max rel err
    2.6e-3 end-to-end vs gate 2e-2), output bf16.  4 MiB HBM traffic/core.

Sharding: pure data parallelism over the batch dim across 8 NeuronCores.
"""

import numpy as np

B = 8388608
NCORES = 8
BC = B // NCORES        # rows per core
P = 128                 # SBUF partitions
NCH = 64                # row chunks per core (x0 chunk c -> partition c,
                        # x1 chunk c -> partition 64+c)
CL = BC // NCH          # chunk length (16384)
NB = 512                # matmul free size (one PSUM bank)
# Per-group free-dim sizes (pairs must match): small edge pairs shorten the
# pipeline ramp (first sigmoid starts sooner) and the serial drain after the
# last sigmoid; big middle pairs amortize per-instruction overhead.
FDS = (1536, 1536, 2048, 2048, 2048, 2048, 2048, 2048, 512, 512)
assert sum(FDS) == CL and all(f % NB == 0 for f in FDS)
assert all(FDS[i] == FDS[i + 1] for i in range(0, len(FDS), 2))


def _sigmoid_np(z):
    out = np.empty_like(z)
    pos = z >= 0
    out[pos] = 1.0 / (1.0 + np.exp(-z[pos]))
    ez = np.exp(z[~pos])
    out[~pos] = ez / (1.0 + ez)
    return out


def _numpy_fallback(x, fc1_tw, fc1_power, fc1_bias, m4_tw, m4_power, m4_bias3):
    """Bit-faithful re-implementation of the reference for degenerate params."""
    x = x.astype(np.float32)
    pw = x[:, None, :] ** fc1_power[None, :, :]
    h = np.sum(fc1_tw[None, :, :, 0] * pw, axis=2) + fc1_bias
    h = _sigmoid_np(h.astype(np.float32))
    x0, x1 = h[:, 0], h[:, 1]
    s1 = m4_tw[0, 0] * x0 ** m4_power[0]
    s2 = m4_tw[1, 0] * x1 ** m4_power[1]
    p1 = m4_tw[2, 0] * x0 ** m4_power[2]
    p2 = m4_tw[3, 0] * x1 ** m4_power[3]
    prod = (s1 + s2 + p1 * p2 + m4_bias3[0])[:, None]
    return _sigmoid_np(prod.astype(np.float32))


def _prep(x, fc1_tw, fc1_power, fc1_bias, m4_tw, m4_power, m4_bias3):
    """Derive scalar constants; return None if this input needs the fallback."""
    w = fc1_tw[:, :, 0].astype(np.float64)
    fb = fc1_bias.astype(np.float64)
    a1, a2 = float(m4_tw[0, 0]), float(m4_tw[1, 0])
    q = float(m4_tw[2, 0]) * float(m4_tw[3, 0])

    if (not np.allclose(fc1_power, 1.0) or not np.allclose(m4_power, 1.0)
            or x.shape != (B, 2) or abs(q) < 1e-6):
        return None

    c0 = a2 / q
    c1 = a1 / q
    cfin = float(m4_bias3[0]) - a1 * a2 / q

    def sig(t):
        return 1.0 / (1.0 + np.exp(-t))

    # v-range over the full hypercube x in [0,1]^2 (h_i monotone in z_i,
    # the product is bilinear in (h0, h1) so extremes are at corners).
    fac = []
    for i, c in ((0, c0), (1, c1)):
        zlo = fb[i] + min(w[i, 0], 0.0) + min(w[i, 1], 0.0)
        zhi = fb[i] + max(w[i, 0], 0.0) + max(w[i, 1], 0.0)
        fac.append((sig(zlo) + c, sig(zhi) + c))
    prods = [f0 * f1 for f0 in fac[0] for f1 in fac[1]]
    vlo = min(q * pr for pr in prods) + cfin
    vhi = max(q * pr for pr in prods) + cfin
    if not (np.isfinite(vlo) and np.isfinite(vhi)) or vhi - vlo < 1e-12:
        return None

    # Minimax-ish linear fit of sigmoid on [vlo, vhi]: secant slope, then
    # the offset that centers the residual.
    alpha = (sig(vhi) - sig(vlo)) / (vhi - vlo)
    ts = np.linspace(vlo, vhi, 20001)
    resid = sig(ts) - alpha * ts
    beta = 0.5 * (resid.max() + resid.min())
    fit_err = 0.5 * (resid.max() - resid.min())
    if fit_err > 4e-3:
        return None  # sigmoid too curved here; use exact fallback

    consts = dict(
        w00=w[0, 0], w01=w[0, 1], b0=float(fb[0]),
        w10=w[1, 0], w11=w[1, 1], b1=float(fb[1]),
        c0=float(c0),
        Aq=float(alpha * q), Bq=float(alpha * q * c1),
        D=float(alpha * cfin + beta),
    )
    return consts


def _build_nc(consts):
    import concourse.bacc as bacc
    import concourse.tile as tile
    from concourse import mybir

    bf16 = mybir.dt.bfloat16
    f8 = mybir.dt.float8e4
    f32 = mybir.dt.float32
    Sig = mybir.ActivationFunctionType.Sigmoid
    MUL = mybir.AluOpType.mult
    ADD = mybir.AluOpType.add

    nc = bacc.Bacc(None, target_bir_lowering=False)
    x8 = nc.dram_tensor("x8", [2, BC], f8, kind="ExternalInput")
    wt = nc.dram_tensor("wt", [P, P], bf16, kind="ExternalInput")
    y = nc.dram_tensor("y", [BC], bf16, kind="ExternalOutput")
    # [128, CL]: partition c = x0 chunk c, partition 64+c = x1 chunk c
    xr = x8[:].rearrange("two (c w) -> (two c) w", c=NCH)
    # Output row r = c*CL + off + n ; the yo tile of pair j holds group 2j
    # on partitions 0-63 and group 2j+1 on partitions 64-127.
    yc = y[:].rearrange("(c w) -> c w", c=NCH)
    FDMAX = max(FDS)

    with tile.TileContext(nc) as tc:
        with tc.tile_pool(name="consts", bufs=1) as cp, \
             tc.tile_pool(name="io", bufs=3) as io, \
             tc.tile_pool(name="ps", bufs=2, space="PSUM") as ps, \
             tc.tile_pool(name="work", bufs=3) as work:
            # Resident block-diagonal weights: first DMA in the queue so
            # LDWEIGHTS (and the first matmul) unblocks as early as possible.
            wtile = cp.tile([P, P], bf16)
            # Weights go on the scalar engine's HWDGE queue so the transfer
            # overlaps the first input load on the sync queue.
            nc.scalar.dma_start(out=wtile, in_=wt[:])
            # Per-partition bias: b0 on parts 0-63, b1 on parts 64-127.
            bt = cp.tile([P, 1], f32)
            nc.vector.memset(bt[0:NCH, :], consts["b0"])
            nc.vector.memset(bt[NCH:P, :], consts["b1"])
            # Warm the ACT sigmoid table set during the first input DMA.
            wz = cp.tile([P, 1], f32)
            nc.vector.memset(wz, 0.0)
            wsg = cp.tile([P, 1], f32)
            nc.scalar.activation(wsg, wz, Sig, bias=bt[:])

            hs = []
            off = 0
            offs = []
            for g, FD in enumerate(FDS):
                xin = io.tile([P, FDMAX], f8, tag="xin", name="xin",
                              bufs=5)[:, :FD]
                nc.sync.dma_start(out=xin, in_=xr[:, off:off + FD])
                z = ps.tile([P, FDMAX], f32, tag="z", name="z")[:, :FD]
                for j in range(FD // NB):
                    nc.tensor.matmul(out=z[:, j * NB:(j + 1) * NB],
                                     lhsT=wtile[:],
                                     rhs=xin[:, j * NB:(j + 1) * NB],
                                     start=True, stop=True)
                h = work.tile([P, FDMAX], bf16, tag="h", name="h",
                              bufs=4)[:, :FD]
                nc.scalar.activation(h, z[:], Sig, bias=bt[:])
                hs.append(h)
                offs.append(off)
                off += FD

                if g % 2 == 1:
                    ha, hb = hs[-2], hs[-1]
                    # Pair the two groups into full-width [128, FD] tiles:
                    # parts 0-63 <- group a, parts 64-127 <- group b, with
                    # halves rebased where needed (single-tensor-input ops
                    # may rebase partitions).
                    t0 = work.tile([P, FDMAX], bf16, tag="t0", name="t0",
                                   bufs=2)[:, :FD]
                    nc.vector.tensor_scalar_add(t0[0:NCH, :], ha[0:NCH, :],
                                                consts["c0"])
                    nc.vector.tensor_scalar_add(t0[NCH:P, :], hb[0:NCH, :],
                                                consts["c0"])
                    g1 = work.tile([P, FDMAX], bf16, tag="g1", name="g1",
                                   bufs=2)[:, :FD]
                    nc.vector.tensor_scalar(g1[0:NCH, :], ha[NCH:P, :],
                                            consts["Aq"], consts["Bq"],
                                            MUL, ADD)
                    nc.vector.tensor_scalar(g1[NCH:P, :], hb[NCH:P, :],
                                            consts["Aq"], consts["Bq"],
                                            MUL, ADD)
                    yo = io.tile([P, FDMAX], bf16, tag="yo", name="yo",
                                 bufs=3)[:, :FD]
                    # Outputs on the sync HWDGE ring too: SWDGE (gpsimd)
                    # costs ~1.5us issue per DMA plus a long per-packet
                    # semaphore drain at kernel end.  For the final pair,
                    # split the product per half so the first output DMA
                    # fires before the second half computes (shorter drain).
                    if g == len(FDS) - 1:
                        nc.vector.tensor_tensor(out=yo[NCH:P, :],
                                                in0=t0[NCH:P, :],
                                                in1=g1[NCH:P, :], op=MUL)
                        nc.sync.dma_start(out=yc[:, offs[-1]:offs[-1] + FD],
                                          in_=yo[NCH:P, :])
                        nc.vector.tensor_tensor(out=yo[0:NCH, :],
                                                in0=t0[0:NCH, :],
                                                in1=g1[0:NCH, :], op=MUL)
                        nc.sync.dma_start(out=yc[:, offs[-2]:offs[-2] + FD],
                                          in_=yo[0:NCH, :])
                    else:
                        nc.vector.tensor_tensor(out=yo[:], in0=t0[:],
                                                in1=g1[:], op=MUL)
                        nc.sync.dma_start(out=yc[:, offs[-2]:offs[-2] + FD],
                                          in_=yo[0:NCH, :])
                        nc.sync.dma_start(out=yc[:, offs[-1]:offs[-1] + FD],
                                          in_=yo[NCH:P, :])

    nc.finalize()
    return nc


def _make_in_maps(x, consts):
    import ml_dtypes
    x0 = x[:, 0].astype(ml_dtypes.float8_e4m3).reshape(NCORES, BC)
    x1 = x[:, 1].astype(ml_dtypes.float8_e4m3).reshape(NCORES, BC)
    xc = np.empty((NCORES, 2, BC), dtype=ml_dtypes.float8_e4m3)
    xc[:, 0, :] = x0
    xc[:, 1, :] = x1
    # Block-diagonal lhsT [K=128, M=128]: out[m] = sum_k wt[k, m] * in[k].
    # m<64:  z0 chunk m  = w00*x0_m + w01*x1_m
    # m>=64: z1 chunk m' = w10*x0_m' + w11*x1_m'
    wtm = np.zeros((P, P), dtype=ml_dtypes.bfloat16)
    for m in range(NCH):
        wtm[m, m] = consts["w00"]
        wtm[NCH + m, m] = consts["w01"]
        wtm[m, NCH + m] = consts["w10"]
        wtm[NCH + m, NCH + m] = consts["w11"]
    return [{"x8": xc[c], "wt": wtm} for c in range(NCORES)]


def _postprocess(res, consts):
    # y rows per core follow r = c*CL + k*FD + n with the natural flat
    # layout, so a straight concat + reshape restores order.
    yo = np.concatenate([res.results[c]["y"] for c in range(NCORES)], axis=0)
    out = yo.astype(np.float32) + np.float32(consts["D"])
    return out.reshape(B, 1)


def kernel(x, fc1_tw, fc1_power, fc1_bias, m4_tw, m4_power, m4_bias3):
    x = np.ascontiguousarray(x, dtype=np.float32)
    fc1_tw = np.asarray(fc1_tw, dtype=np.float32)
    fc1_power = np.asarray(fc1_power, dtype=np.float32)
    fc1_bias = np.asarray(fc1_bias, dtype=np.float32)
    m4_tw = np.asarray(m4_tw, dtype=np.float32)
    m4_power = np.asarray(m4_power, dtype=np.float32)
    m4_bias3 = np.asarray(m4_bias3, dtype=np.float32)

    consts = _prep(x, fc1_tw, fc1_power, fc1_bias, m4_tw, m4_power, m4_bias3)
    if consts is None:
        return _numpy_fallback(x, fc1_tw, fc1_power, fc1_bias,
                               m4_tw, m4_power, m4_bias3)

    from concourse.bass_utils import run_bass_kernel_spmd

    nc = _build_nc(consts)
    res = run_bass_kernel_spmd(nc, _make_in_maps(x, consts),
                               core_ids=list(range(NCORES)))
    return _postprocess(res, consts)
